# revision 30
# baseline (speedup 1.0000x reference)
"""Cross-attention Trainium2 kernel for nn_CrossAttention_37495064494692.

B=8 batches sharded 1/core across 8 NeuronCores (data parallel).
Per core: full cross-attention for one batch element in
feature-on-partitions ("transposed") layouts. Matmuls in bf16 (fp32
PSUM accumulation); softmax math fp32.

v4 structure:
  * Wt folded into Wkv on the host: KT = Wtk^T @ guideT + bk2,
    V = guideT^T @ Wtv + bv2 (no gT intermediate; fewer MACs).
  * attention_mask folded multiplicatively into V rows and into the
    denominator matmul lhsT (exact for 0/1 masks) -- exp needs no
    per-L-tile bias, so score psum tiles span two banks [128,1024] and
    exp runs as one activation per 2 L-tiles.
  * PV packs two heads per matmul (M=128); softmax denominators come
    from col-packed M=1 matmuls (4 heads -> one [97,512] psum tile at
    partitions 0/32/64/96), so only 4 exact DVE reciprocals per chunk.
  * normalization: gpsimd partition_broadcast of each recip row ->
    [128,512], then per-head base-aligned DVE multiplies psum*bc -> OT.
  * global software pipeline over slots (c, hp); per slot the PE order
    is scores-g0 | QT(c+1) | scores-g1 | PV(c,hp-1) | outproj(c-1,hp) |
    denom(c,hp), which hides ACT exp and the normalize chain.
"""
import sys

sys.path.insert(0, "/opt/trn_rl_repo")

import ml_dtypes
import numpy as np

import concourse.bacc as bacc
import concourse.bass as bass
import concourse.tile as tile
from concourse import mybir
from concourse.bass_utils import run_bass_kernel_spmd

F32 = mybir.dt.float32
BF16 = mybir.dt.bfloat16
FP8 = mybir.dt.float8e4
MMDT = BF16
NPDT = ml_dtypes.bfloat16
NP8 = ml_dtypes.float8_e4m3fn

B, S, L = 8, 2048, 512
E, TE, H = 1024, 768, 16
D = E // H
SCALE = D ** -0.5

SC = 512              # s-chunk width
N_SC = S // SC        # 4 s-chunks
N_E = E // 128        # 8 E-chunks
N_TE = TE // 128      # 6 TE-chunks
N_LT = L // 128       # 4 L-tiles
HP = H // 2           # 8 head pairs

TRACE = False
DEBUG = False
_CACHED_NC = None


def build_nc():
    nc = bacc.Bacc()

    queryT = nc.declare_dram_parameter("queryT", [E, S], MMDT, isOutput=False)
    guideT = nc.declare_dram_parameter("guideT", [TE, L], MMDT, isOutput=False)
    Wq = nc.declare_dram_parameter("Wq", [E, E], MMDT, isOutput=False)
    Wtk = nc.declare_dram_parameter("Wtk", [TE, E], MMDT, isOutput=False)
    Wtv = nc.declare_dram_parameter("Wtv", [TE, E], MMDT, isOutput=False)
    Wo = nc.declare_dram_parameter("Wo", [E, E], MMDT, isOutput=False)
    bq = nc.declare_dram_parameter("bq", [E], F32, isOutput=False)
    bk2 = nc.declare_dram_parameter("bk2", [E], F32, isOutput=False)
    bv2_r = nc.declare_dram_parameter("bv2_r", [E], MMDT, isOutput=False)
    bo = nc.declare_dram_parameter("bo", [1, E], F32, isOutput=False)
    mmul = nc.declare_dram_parameter("mmul", [L], F32, isOutput=False)
    out = nc.declare_dram_parameter("out", [S, E], F32, isOutput=True)
    if DEBUG:
        dbg_dn = nc.declare_dram_parameter("dbg_dn", [97, SC], F32, isOutput=True)
        dbg_rc = nc.declare_dram_parameter("dbg_rc", [97, SC], F32, isOutput=True)
        dbg_pv = nc.declare_dram_parameter("dbg_pv", [128, SC], F32, isOutput=True)
        dbg_ot = nc.declare_dram_parameter("dbg_ot", [128, SC], F32, isOutput=True)
        dbg_at = nc.declare_dram_parameter("dbg_at", [128, 2 * SC], F32,
                                           isOutput=True)

    Exp = mybir.ActivationFunctionType.Exp

    with tile.TileContext(nc) as tc:
        with (
            tc.tile_pool(name="res", bufs=1) as res,
            tc.tile_pool(name="psS", bufs=1, space="PSUM") as psS,
            tc.tile_pool(name="psA", bufs=2, space="PSUM") as psA,
            tc.tile_pool(name="psV", bufs=1, space="PSUM") as psV,
            tc.tile_pool(name="psD", bufs=1, space="PSUM") as psD,
            tc.tile_pool(name="io", bufs=2) as io,
            tc.tile_pool(name="stp", bufs=4) as stp,
        ):
            # ---- resident small tensors ----
            bq_sb = res.tile([128, N_E], F32, tag="bq")
            bk_sb = res.tile([128, N_E], F32, tag="bk")
            mm_sb = res.tile([128, N_LT], F32, tag="mm")
            nc.sync.dma_start(out=bq_sb, in_=bq.rearrange("(t p) -> p t", p=128))
            nc.sync.dma_start(out=bk_sb, in_=bk2.rearrange("(t p) -> p t", p=128))
            nc.sync.dma_start(out=mm_sb, in_=mmul.rearrange("(t p) -> p t", p=128))
            mcol_b = res.tile([128, N_LT], MMDT, tag="mcb")
            nc.scalar.copy(mcol_b, mm_sb)
            bo_row = res.tile([1, E], F32, tag="bor")
            nc.sync.dma_start(out=bo_row, in_=bo[:, :])
            bo_bc = res.tile([128, E], F32, tag="bo")
            nc.gpsimd.partition_broadcast(bo_bc, bo_row, channels=128)
            bv_row = res.tile([1, E], MMDT, tag="bvr")
            nc.sync.dma_start(out=bv_row, in_=bv2_r.rearrange("(one f) -> one f", one=1))
            ones_f = res.tile([1, 512], F32, tag="ones_f")
            ones_r = res.tile([1, 512], MMDT, tag="ones_r")
            nc.vector.memset(ones_f, 1.0)
            nc.scalar.copy(ones_r, ones_f)
            # ~7us of dummy matmuls to lift the HAM clock gate while the
            # weight DMAs stream in
            warm = psA.tile([128, SC], F32, tag="acc")
            for _ in range(16):
                nc.tensor.matmul(warm, lhsT=ones_r[:, 0:128], rhs=ones_r,
                                 start=True, stop=True)

            # ---- weights: Wq + first query chunks first, guide path next ----
            Wq_sb = [res.tile([128, E], MMDT, tag=f"wq{e}", name=f"wq{e}")
                     for e in range(N_E)]
            for e in range(N_E):
                nc.sync.dma_start(out=Wq_sb[e], in_=Wq[e * 128:(e + 1) * 128, :])
            qT = {}
            qT[0] = [io.tile([128, SC], MMDT, tag=f"qin{e}", name=f"qin{e}_0")
                     for e in range(N_E)]
            for e in range(N_E):
                nc.sync.dma_start(out=qT[0][e], in_=queryT[e * 128:(e + 1) * 128, 0:SC])
            g_in = [res.tile([128, L], MMDT, tag=f"gin{t}", name=f"gin{t}")
                    for t in range(N_TE)]
            for t in range(N_TE):
                nc.sync.dma_start(out=g_in[t], in_=guideT[t * 128:(t + 1) * 128, :])
            Wtk_sb = [res.tile([128, E], MMDT, tag=f"wtk{t}", name=f"wtk{t}")
                      for t in range(N_TE)]
            for t in range(N_TE):
                nc.sync.dma_start(out=Wtk_sb[t], in_=Wtk[t * 128:(t + 1) * 128, :])
            Wtv_sb = [res.tile([128, E], MMDT, tag=f"wtv{t}", name=f"wtv{t}")
                      for t in range(N_TE)]
            for t in range(N_TE):
                nc.sync.dma_start(out=Wtv_sb[t], in_=Wtv[t * 128:(t + 1) * 128, :])
            qT[1] = [io.tile([128, SC], MMDT, tag=f"qin{e}", name=f"qin{e}_1")
                     for e in range(N_E)]
            for e in range(N_E):
                nc.sync.dma_start(out=qT[1][e],
                                  in_=queryT[e * 128:(e + 1) * 128, SC:2 * SC])
            Wo_sb = [res.tile([128, E], MMDT, tag=f"wo{e}", name=f"wo{e}")
                     for e in range(N_E)]
            for e in range(N_E):
                nc.sync.dma_start(out=Wo_sb[e], in_=Wo[e * 128:(e + 1) * 128, :])

            # ---- long-lived activations ----
            KT = [res.tile([128, L], MMDT, tag=f"KT{j}", name=f"KT{j}")
                  for j in range(N_E)]
            # V: head h at cols h*64..(h+1)*64, masked L rows zeroed
            Vt = [res.tile([128, E], MMDT, tag=f"V{lt}", name=f"V{lt}")
                  for lt in range(N_LT)]

            # ================= prologue =================
            def qt_mms(ps, c, j):
                for e in range(N_E):
                    nc.tensor.matmul(
                        ps, lhsT=Wq_sb[e][:, j * 128:(j + 1) * 128], rhs=qT[c][e],
                        start=(e == 0), stop=(e == N_E - 1),
                    )

            QT = {}
            QT[0] = [io.tile([128, SC], MMDT, tag=f"QT{j}", name=f"QT{j}_0")
                     for j in range(N_E)]
            for j in range(N_E):
                ps = psA.tile([128, SC], F32, tag="acc")
                qt_mms(ps, 0, j)
                nc.vector.tensor_scalar_add(QT[0][j], ps, bq_sb[:, j:j + 1])

            # KT = Wtk^T @ guideT + bk2
            for j in range(N_E):
                ps = psA.tile([128, SC], F32, tag="acc")
                for t in range(N_TE):
                    nc.tensor.matmul(
                        ps, lhsT=Wtk_sb[t][:, j * 128:(j + 1) * 128], rhs=g_in[t],
                        start=(t == 0), stop=(t == N_TE - 1),
                    )
                nc.vector.tensor_scalar_add(KT[j], ps, bk_sb[:, j:j + 1])

            # V = guideT^T @ Wtv + bv2, then mask-zero rows
            for lt in range(N_LT):
                for half in range(2):
                    ps = psA.tile([128, SC], F32, tag="acc")
                    for t in range(N_TE):
                        nc.tensor.matmul(
                            ps, lhsT=g_in[t][:, lt * 128:(lt + 1) * 128],
                            rhs=Wtv_sb[t][:, half * SC:(half + 1) * SC],
                            start=(t == 0), stop=False,
                        )
                    nc.tensor.matmul(
                        ps, lhsT=ones_r[:, 0:128],
                        rhs=bv_row[:, half * SC:(half + 1) * SC],
                        start=False, stop=True,
                    )
                    nc.vector.tensor_copy(Vt[lt][:, half * SC:(half + 1) * SC], ps)
                nc.vector.tensor_scalar_mul(Vt[lt], Vt[lt], mm_sb[:, lt:lt + 1])

            # ================= pipelined main loop =================
            OT = {}
            att = {}
            dps = {}    # psD tile per (c, hp//2)
            bcs = {}    # broadcast recip tiles per (c, hp)

            def issue_scores(c, hp, g):
                grp = [psS.tile([128, 2 * SC], F32, tag=f"sc{u}",
                                name=f"sc{u}_{c}_{hp}_{g}")
                       for u in range(2)]
                for lt_i in range(2):
                    lt = 2 * g + lt_i
                    for u in range(2):
                        rows = slice(u * 64, (u + 1) * 64)
                        nc.tensor.matmul(
                            grp[u][:, lt_i * SC:(lt_i + 1) * SC],
                            lhsT=KT[hp][rows, lt * 128:(lt + 1) * 128],
                            rhs=QT[c][hp][rows, :],
                            start=True, stop=True,
                        )
                for u in range(2):
                    a = io.tile([128, 2 * SC], MMDT, tag=f"at{u}{g}",
                                name=f"at{u}{g}_{c}_{hp}")
                    nc.scalar.activation(a, grp[u], Exp, scale=SCALE)
                    att[(c, hp, u, g)] = a

            def issue_qt(c, j):
                ps = psA.tile([128, SC], F32, tag="acc")
                qt_mms(ps, c, j)
                nc.vector.tensor_scalar_add(QT[c][j], ps, bq_sb[:, j:j + 1])

            def issue_denom(c, hp):
                # col-packed M=1 sums of att over L; 4 heads per psD tile
                grp_i = hp // 2
                if hp % 2 == 0:
                    dps[(c, grp_i)] = psD.tile([97, SC], F32, tag="dn",
                                               name=f"dn_{c}_{grp_i}")
                dn = dps[(c, grp_i)]
                for lt in range(N_LT):
                    g, lt_i = lt // 2, lt % 2
                    for u in range(2):
                        h = 2 * hp + u
                        k = 32 * (h % 4)
                        nc.tensor.matmul(
                            dn[k:k + 1, :],
                            lhsT=mcol_b[:, lt:lt + 1],
                            rhs=att[(c, hp, u, g)][:, lt_i * SC:(lt_i + 1) * SC],
                            start=(lt == 0), stop=(lt == N_LT - 1),
                            tile_position=(0, k),
                            skip_group_check=True,
                        )
                if hp % 2 == 1:
                    rc = stp.tile([97, SC], F32, tag="rc", name=f"rc_{c}_{grp_i}",
                                  bufs=2)
                    if DEBUG and c == 0 and hp == 1:
                        dsb = stp.tile([97, SC], F32, tag="dbgd", name="dbgd", bufs=1)
                        nc.vector.tensor_copy(dsb, dn)
                        nc.sync.dma_start(out=dbg_dn[:, :], in_=dsb)
                    nc.vector.reciprocal(rc, dn)
                    if DEBUG and c == 0 and hp == 1:
                        nc.sync.dma_start(out=dbg_rc[:, :], in_=rc)
                    # stage each recip row at partition 0 (partition_broadcast
                    # reads garbage from nonzero source partitions), then
                    # broadcast; consumed by norm at lag-2
                    for hh in (hp - 1, hp):
                        for u in range(2):
                            h = 2 * hh + u
                            rck = stp.tile([1, SC], F32, tag="rck",
                                           name=f"rck{h}_{c}")
                            nc.sync.dma_start(
                                out=rck, in_=rc[32 * (h % 4):32 * (h % 4) + 1, :])
                            bc = stp.tile([128, SC], F32, tag="bc",
                                          name=f"bc{h}_{c}")
                            nc.gpsimd.partition_broadcast(bc, rck, channels=128)
                            bcs[(c, hh, u)] = bc

            def issue_pv(c, hp):
                # two col-packed M=64 matmuls per L-tile (one per head),
                # accumulated over lt; then stage unnormalized OT via ACT
                pv = psV.tile([128, SC], F32, tag="pv", name=f"pv{hp}_{c}")
                for lt in range(N_LT):
                    g, lt_i = lt // 2, lt % 2
                    nc.tensor.matmul(
                        pv[0:64, :],
                        lhsT=Vt[lt][:, hp * 128:hp * 128 + 64],
                        rhs=att[(c, hp, 0, g)][:, lt_i * SC:(lt_i + 1) * SC],
                        start=(lt == 0), stop=(lt == N_LT - 1),
                        skip_group_check=True,
                    )
                    nc.tensor.matmul(
                        pv[64:128, :],
                        lhsT=Vt[lt][:, hp * 128 + 64:(hp + 1) * 128],
                        rhs=att[(c, hp, 1, g)][:, lt_i * SC:(lt_i + 1) * SC],
                        start=(lt == 0), stop=(lt == N_LT - 1),
                        skip_group_check=True,
                    )
                if DEBUG and c == 0 and hp == 0:
                    psb = stp.tile([128, SC], F32, tag="dbgp", name="dbgp", bufs=1)
                    nc.vector.tensor_copy(psb, pv)
                    nc.sync.dma_start(out=dbg_pv[:, :], in_=psb)
                    asb = stp.tile([128, 2 * SC], F32, tag="dbga", name="dbga", bufs=1)
                    nc.vector.tensor_copy(asb, att[(c, hp, 0, 0)])
                    nc.sync.dma_start(out=dbg_at[:, :], in_=asb)
                nc.scalar.copy(OT[c][hp], pv)

            def issue_norm(c, hp):
                # in-place normalize of staged OT (bcasts ready since lag-2)
                for u in range(2):
                    rows = slice(u * 64, (u + 1) * 64)
                    bc = bcs.pop((c, hp, u))
                    nc.vector.tensor_mul(
                        OT[c][hp][rows, :], OT[c][hp][rows, :], bc[rows, :])
                if DEBUG and c == 0 and hp == 0:
                    osb = stp.tile([128, SC], F32, tag="dbgo", name="dbgo", bufs=1)
                    nc.vector.tensor_copy(osb, OT[c][hp])
                    nc.sync.dma_start(out=dbg_ot[:, :], in_=osb)

            def issue_outproj(c, grp_i):
                st, half = grp_i // 2, grp_i % 2
                ps = psA.tile([128, SC], F32, tag="acc")
                for j in range(N_E):
                    nc.tensor.matmul(
                        ps, lhsT=OT[c][j][:, st * 128:(st + 1) * 128],
                        rhs=Wo_sb[j][:, half * SC:(half + 1) * SC],
                        start=(j == 0), stop=(j == N_E - 1),
                    )
                ob = stp.tile([128, SC], F32, tag="ob", name=f"ob{grp_i}_{c}")
                nc.vector.tensor_add(ob, ps, bo_bc[:, half * SC:(half + 1) * SC])
                nc.sync.dma_start(
                    out=out[c * SC + st * 128: c * SC + (st + 1) * 128,
                            half * SC:(half + 1) * SC],
                    in_=ob,
                )

            for c in range(N_SC + 2):
                if c < N_SC:
                    OT[c] = [io.tile([128, SC], MMDT, tag=f"OT{j}",
                                     name=f"OT{j}_{c}", bufs=3)
                             for j in range(N_E)]
                if c + 2 < N_SC:
                    cn = c + 2
                    qT[cn] = [io.tile([128, SC], MMDT, tag=f"qin{e}",
                                      name=f"qin{e}_{cn}")
                              for e in range(N_E)]
                    for e in range(N_E):
                        nc.sync.dma_start(
                            out=qT[cn][e],
                            in_=queryT[e * 128:(e + 1) * 128,
                                       cn * SC:(cn + 1) * SC])
                if c + 1 < N_SC:
                    QT[c + 1] = [io.tile([128, SC], MMDT, tag=f"QT{j}",
                                         name=f"QT{j}_{c + 1}")
                                 for j in range(N_E)]
                for hp in range(HP):
                    s = c * HP + hp
                    if c < N_SC:
                        issue_scores(c, hp, 0)
                    if c + 1 < N_SC:
                        issue_qt(c + 1, hp)
                    if c < N_SC:
                        issue_scores(c, hp, 1)
                    t = s - 1          # PV lags one slot
                    if t >= 0 and t // HP < N_SC:
                        issue_pv(t // HP, t % HP)
                    t2 = s - 2         # normalize lags two slots
                    if t2 >= 0 and t2 // HP < N_SC:
                        issue_norm(t2 // HP, t2 % HP)
                    t3 = s - 10        # out-projection skewed two more
                    if t3 >= 0 and t3 // HP < N_SC:
                        issue_outproj(t3 // HP, t3 % HP)
                    if c < N_SC:
                        issue_denom(c, hp)

    if not nc.is_finalized():
        nc.finalize()
    return nc


def kernel(query, guide_vector, attention_mask, Wt, bt, Wq, bq, Wkv, bkv, Wo, bo):
    global _CACHED_NC
    query = np.asarray(query, dtype=np.float32)
    guide_vector = np.asarray(guide_vector, dtype=np.float32)
    attention_mask = np.asarray(attention_mask)
    Wt = np.asarray(Wt, dtype=np.float32)
    bt = np.asarray(bt, dtype=np.float32)
    bq = np.asarray(bq, dtype=np.float32)
    Wkv = np.asarray(Wkv, dtype=np.float32)
    bkv = np.asarray(bkv, dtype=np.float32)
    bo = np.asarray(bo, dtype=np.float32)

    # fold the text projection into the kv projection (exact algebra)
    Wtk = Wt @ Wkv[:, :E]
    Wtv = Wt @ Wkv[:, E:]
    bk2 = bt @ Wkv[:, :E] + bkv[:E]
    bv2 = bt @ Wkv[:, E:] + bkv[E:]

    Wq_m = np.asarray(Wq, dtype=np.float32).astype(NPDT)
    Wo_m = np.asarray(Wo, dtype=np.float32).astype(NPDT)
    Wtk_m = Wtk.astype(NPDT)
    Wtv_m = Wtv.astype(NPDT)
    bv_m = bv2.astype(NPDT)

    if _CACHED_NC is None:
        _CACHED_NC = build_nc()
    nc = _CACHED_NC

    mm = attention_mask.astype(np.float32)
    in_maps = []
    for b in range(B):
        in_maps.append({
            "queryT": np.ascontiguousarray(query[b].T).astype(NPDT),
            "guideT": np.ascontiguousarray(guide_vector[b].T).astype(NPDT),
            "Wq": Wq_m, "Wtk": Wtk_m, "Wtv": Wtv_m, "Wo": Wo_m,
            "bq": bq, "bk2": bk2, "bv2_r": bv_m,
            "bo": bo.reshape(1, E), "mmul": mm[b],
        })
    res = run_bass_kernel_spmd(nc, in_maps, list(range(B)), trace=TRACE)
    global _LAST_RES
    _LAST_RES = res
    if TRACE:
        kernel.last_exec_time_ns = res.exec_time_ns
        kernel.last_results = res
    return np.stack([res.results[b]["out"] for b in range(B)])
